# revision 3
# baseline (speedup 1.0000x reference)
"""Trainium2 Bass kernel for nn_Euler: 512-step Euler integration of a
2-layer tanh MLP, data-parallel over 8 NeuronCores (batch 1024 -> 128/core).

Layout per core (hT orientation, state transposed):
  zT = [stateT; uT; ones] (97 partitions x 128 batch), split fp16 hi/lo.
  mm1 (fp16 hi/lo 3-term): psum_h[128, 4*128] = chunks of (z @ [W1;b1]).T
  tanh: ACT psum -> h fp32 SBUF
  mm2 (fp32): diffT = (DT*W2).T @ h chunks + DT*b2, accumulated in PSUM
  update: DVE stateT += diffT; re-split state to fp16 hi/lo for next step.
State is carried in fp32 end-to-end; matmul precision ~1e-5 rel vs fp32.
"""

import numpy as np
from contextlib import ExitStack

B, L, S, U, H = 1024, 512, 64, 32, 512
DT = 0.1
NCORES = 8
BLOC = B // NCORES  # 128
KZ = S + U + 1      # 97 (state + control + bias row)
NCH = H // 128      # 4 H-chunks

_COMPILED = None


def _build(nsteps):
    import concourse.bacc as bacc
    import concourse.tile as tile
    import concourse.mybir as mybir

    F32 = mybir.dt.float32
    F16 = mybir.dt.float16
    TANH = mybir.ActivationFunctionType.Tanh
    ADD = mybir.AluOpType.add
    SUB = mybir.AluOpType.subtract

    nc = bacc.Bacc("TRN2", target_bir_lowering=False, debug=False,
                   num_devices=NCORES)

    s0T_d = nc.dram_tensor("s0T", [S, BLOC], F32, kind="ExternalInput").ap()
    uhi_d = nc.dram_tensor("uhiT", [nsteps, U, BLOC], F16, kind="ExternalInput").ap()
    ulo_d = nc.dram_tensor("uloT", [nsteps, U, BLOC], F16, kind="ExternalInput").ap()
    w1hi_d = nc.dram_tensor("w1hi", [KZ, H], F16, kind="ExternalInput").ap()
    w1lo_d = nc.dram_tensor("w1lo", [KZ, H], F16, kind="ExternalInput").ap()
    w2_d = nc.dram_tensor("w2", [NCH, 128, S], F32, kind="ExternalInput").ap()
    b2_d = nc.dram_tensor("b2row", [1, S], F32, kind="ExternalInput").ap()
    out_d = nc.dram_tensor("outT", [nsteps, S, BLOC], F32, kind="ExternalOutput").ap()

    with tile.TileContext(nc) as tc, ExitStack() as ctx:
        cpool = ctx.enter_context(tc.tile_pool(name="const", bufs=1))
        spool = ctx.enter_context(tc.tile_pool(name="state", bufs=1))
        hpool = ctx.enter_context(tc.tile_pool(name="h", bufs=2))
        upool = ctx.enter_context(tc.tile_pool(name="u", bufs=4))
        opool = ctx.enter_context(tc.tile_pool(name="outs", bufs=4))
        pp_h = ctx.enter_context(tc.tile_pool(name="ps_h", bufs=2, space="PSUM"))
        pp_d = ctx.enter_context(tc.tile_pool(name="ps_d", bufs=2, space="PSUM"))

        # --- static weights/constants ---
        w1hi = cpool.tile([KZ, H], F16)
        w1lo = cpool.tile([KZ, H], F16)
        w2 = cpool.tile([128, NCH * S], F32)
        b2r = cpool.tile([1, S], F32)
        ones = cpool.tile([1, BLOC], F32)
        nc.sync.dma_start(w1hi[:, :], w1hi_d[:, :])
        nc.sync.dma_start(w1lo[:, :], w1lo_d[:, :])
        for j in range(NCH):
            nc.sync.dma_start(w2[:, j * S:(j + 1) * S], w2_d[j, :, :])
        nc.sync.dma_start(b2r[:, :], b2_d[:, :])
        nc.vector.memset(ones[:, :], 1.0)

        # --- double-buffered z (hi/lo) and state tiles ---
        zhi = [spool.tile([KZ, BLOC], F16, tag=f"zhi{i}", name=f"zhi{i}") for i in range(2)]
        zlo = [spool.tile([KZ, BLOC], F16, tag=f"zlo{i}", name=f"zlo{i}") for i in range(2)]
        sT = [spool.tile([S, BLOC], F32, tag=f"sT{i}", name=f"sT{i}") for i in range(2)]
        for i in range(2):
            nc.vector.memset(zhi[i][S + U:KZ, :], 1.0)   # bias row (hi = 1.0)
            nc.vector.memset(zlo[i][S + U:KZ, :], 0.0)   # bias row (lo = 0)

        # --- prologue: seed state buffers from s0 ---
        nc.sync.dma_start(sT[0][:, :], s0T_d[:, :])
        nc.vector.tensor_copy(zhi[0][:S, :], sT[0][:, :])
        nc.vector.tensor_tensor(zlo[0][:S, :], sT[0][:, :], zhi[0][:S, :], SUB)
        nc.sync.dma_start(zhi[0][S:S + U, :], uhi_d[0, :, :])
        nc.sync.dma_start(zlo[0][S:S + U, :], ulo_d[0, :, :])

        for t in range(nsteps):
            X = t % 2
            Y = (t + 1) % 2
            # mm1: 12 fp16 matmuls -> psum_h (hT chunks)
            ph = pp_h.tile([128, H], F32)
            for j in range(NCH):
                o = ph[:, j * 128:(j + 1) * 128]
                wj = slice(j * 128, (j + 1) * 128)
                nc.tensor.matmul(o, w1hi[:, wj], zhi[X][:, :], start=True, stop=False)
                nc.tensor.matmul(o, w1hi[:, wj], zlo[X][:, :], start=False, stop=False)
                nc.tensor.matmul(o, w1lo[:, wj], zhi[X][:, :], start=False, stop=True)
            # tanh (one ACT instruction over all chunks)
            h = hpool.tile([128, H], F32)
            nc.scalar.activation(h[:, :], ph[:, :], TANH)
            # mm2: fp32, accumulate 4 chunks + bias row
            pd = pp_d.tile([128, BLOC], F32)
            nc.tensor.matmul(pd[:S, :], b2r[:, :], ones[:, :], start=True, stop=False)
            for j in range(NCH):
                nc.tensor.matmul(
                    pd[:S, :], w2[:, j * S:(j + 1) * S],
                    h[:, j * 128:(j + 1) * 128],
                    start=False, stop=(j == NCH - 1),
                )
            # state update + re-split (fp32 carried state)
            nc.vector.tensor_tensor(sT[Y][:, :], sT[X][:, :], pd[:S, :], ADD)
            nc.vector.tensor_copy(zhi[Y][:S, :], sT[Y][:, :])
            nc.vector.tensor_tensor(zlo[Y][:S, :], sT[Y][:, :], zhi[Y][:S, :], SUB)
            # next-step control inputs
            if t + 1 < nsteps:
                nc.sync.dma_start(zhi[Y][S:S + U, :], uhi_d[t + 1, :, :])
                nc.sync.dma_start(zlo[Y][S:S + U, :], ulo_d[t + 1, :, :])
            # stream out new state (sT[Y] is not rewritten until step t+2)
            nc.sync.dma_start(out_d[t, :, :], sT[Y][:, :])

    nc.compile()
    return nc


def _prep_inputs(initial_state, control_inputs, W1, b1, W2, b2, nsteps):
    f32 = np.float32
    f16 = np.float16
    W1b = np.concatenate([W1.astype(f32), b1.astype(f32)[None, :]], axis=0)  # (97, 512)
    w1hi = W1b.astype(f16)
    w1lo = (W1b - w1hi.astype(f32)).astype(f16)
    W2s = (W2.astype(f32) * f32(DT)).reshape(NCH, 128, S).astype(f32)
    b2r = (b2.astype(f32) * f32(DT))[None, :]

    in_maps = []
    for c in range(NCORES):
        sl = slice(c * BLOC, (c + 1) * BLOC)
        s0T = np.ascontiguousarray(initial_state[sl].astype(f32).T)          # (S, BLOC)
        uT = np.ascontiguousarray(
            control_inputs[sl, :nsteps].astype(f32).transpose(1, 2, 0))      # (L, U, BLOC)
        uhi = uT.astype(f16)
        ulo = (uT - uhi.astype(f32)).astype(f16)
        in_maps.append({
            "s0T": s0T, "uhiT": uhi, "uloT": ulo,
            "w1hi": w1hi, "w1lo": w1lo, "w2": W2s, "b2row": b2r,
        })
    return in_maps


def kernel(initial_state, control_inputs, W1, b1, W2, b2, nsteps=L):
    global _COMPILED
    if _COMPILED is None or _COMPILED[1] != nsteps:
        _COMPILED = (_build(nsteps), nsteps)
    nc = _COMPILED[0]

    from concourse.bass_utils import run_bass_kernel_spmd
    in_maps = _prep_inputs(initial_state, control_inputs, W1, b1, W2, b2, nsteps)
    res = run_bass_kernel_spmd(nc, in_maps, list(range(NCORES)))
    out = np.empty((B, nsteps, S), np.float32)
    for c in range(NCORES):
        outT = res.results[c]["outT"]                    # (L, S, BLOC)
        out[c * BLOC:(c + 1) * BLOC] = outT.transpose(2, 0, 1)
    return out
